# revision 13
# baseline (speedup 1.0000x reference)
"""Two-layer GCN (PyG GCNConv-style) on 8 Trainium2 NeuronCores.

Strategy: nodes are partitioned across the 8 cores (load-balanced into
128-row tiles by in-degree), edges partitioned by destination node so
the segment-sum is local.  Each layer is transform-first: local GEMM
(h = x @ W, scaled by dinv[src]), AllGather of the bf16 transformed
features, then a local gather + segment-sum over incoming edges.

The segment-sum runs on the TensorEngine: for each destination tile of
128 nodes, its incoming edges (chunked by 128) are bulk-gathered with
dma_gather into SBUF [128edges x F] per chunk (bf16, halving DMA), and
contracted with a one-hot matrix S [128edges x 128dst] whose nonzeros
carry dinv[dst].  S is host-built and streamed as dense bf16 (the DMA
engines have slack; building S on the vector engine was measured to be
the bottleneck).  The gathers are spread round-robin over 4 SWDGE
queues (num_swdge_queues=4), which parallelizes the per-index Q7
descriptor-generation ucode (~8.7ns/idx serial, measured) about 4x -
this, not bytes, is the dominant cost of dma_gather.

Layer-1 aggregation is computed TRANSPOSED (psxT[f,d] = sum_k G_k^T @
S_k) so the relu output lands directly in lhsT layout for the layer-2
GEMM - no on-device transposes at all.  norm = dinv[src]*dinv[dst] is
factored: dinv[src] is folded into the AllGathered features (free, at
the GEMM psum->sbuf copy), dinv[dst] into the S nonzeros.  Self-loops
use a plain contiguous DMA of the local tile plus a diagonal S block
built by the same machinery (dst=iota, val=dinv).

dma_gather takes int16 row indices, so the gathered table is addressed
through two overlapping <=32767-row windows.
"""

import numpy as np

P = 128
N_CORES = 8
WINDOW_CAP = 32512  # dma_gather int16 window (multiple of 128, <= 32767)

_prog_cache = {}


# ---------------------------------------------------------------- host side


def _bf16(a):
    import ml_dtypes

    return np.asarray(a, dtype=ml_dtypes.bfloat16)


def _preprocess(x, edge_index):
    """Partition nodes/edges, build per-core device arrays."""
    x = np.ascontiguousarray(np.asarray(x, dtype=np.float32))
    ei = np.asarray(edge_index)
    N, IN = x.shape

    src = ei[0].astype(np.int64)
    dst = ei[1].astype(np.int64)

    deg = 1 + np.bincount(dst, minlength=N)  # with self loop, >= 1
    dinv = (1.0 / np.sqrt(deg.astype(np.float64))).astype(np.float32)

    npc_nodes = -(-N // N_CORES)
    T = -(-npc_nodes // P)  # dst tiles per core
    NPC = T * P  # node slots per core
    n_tiles = N_CORES * T
    NG = n_tiles * P  # global node slots

    # --- pack nodes into tiles, balancing per-tile in-degree (LPT) ----
    import heapq

    degg = deg - 1  # gathered (non-self) in-degree
    tile_of = np.empty(N, dtype=np.int64)
    pos_of = np.empty(N, dtype=np.int64)
    counts = np.zeros(n_tiles, dtype=np.int64)
    loads = np.zeros(n_tiles, dtype=np.int64)
    order = np.argsort(-degg, kind="stable")
    heap = [(0, t) for t in range(n_tiles)]
    heapq.heapify(heap)
    deg_l = degg[order]
    for i in range(N):
        v = order[i]
        while True:
            load, t = heapq.heappop(heap)
            if counts[t] < P:
                break
        tile_of[v] = t
        pos_of[v] = counts[t]
        counts[t] += 1
        load += int(deg_l[i])
        loads[t] = load
        if counts[t] < P:
            heapq.heappush(heap, (load, t))

    # repair pass: move small nodes off overloaded tiles to reach the
    # ideal chunk count ceil(total/(n_tiles*P)) if possible
    K_ideal = max(1, int(-(-int(degg.sum()) // (n_tiles * P))))
    target = K_ideal * P
    if loads.max() > target:
        by_tile = [[] for _ in range(n_tiles)]
        for i in range(N - 1, -1, -1):  # ascending degree order
            by_tile[tile_of[order[i]]].append(order[i])
        free = [(loads[t], t) for t in range(n_tiles)
                if counts[t] < P and loads[t] < target]
        heapq.heapify(free)
        for t_over in np.flatnonzero(loads > target):
            stack = by_tile[t_over]
            si = 0
            while loads[t_over] > target and si < len(stack) and free:
                v = stack[si]
                si += 1
                d = int(degg[v])
                moved = False
                tried = []
                while free:
                    lo, t2 = heapq.heappop(free)
                    if lo != loads[t2] or counts[t2] >= P:
                        continue  # stale
                    if loads[t2] + d <= target:
                        tile_of[v] = t2
                        pos_of[v] = counts[t2]
                        counts[t2] += 1
                        loads[t2] += d
                        loads[t_over] -= d
                        moved = True
                        if counts[t2] < P and loads[t2] < target:
                            heapq.heappush(free, (loads[t2], t2))
                        break
                    tried.append((lo, t2))
                for it in tried:
                    heapq.heappush(free, it)
                if not moved:
                    break
        # recompute pos_of consistently (holes possible after moves)
        ordv = np.lexsort((np.arange(N), tile_of))
        pos = np.empty(N, dtype=np.int64)
        tt = tile_of[ordv]
        st = np.zeros(n_tiles + 1, dtype=np.int64)
        np.cumsum(np.bincount(tt, minlength=n_tiles), out=st[1:])
        pos[ordv] = np.arange(N) - st[tt]
        pos_of = pos

    K = max(1, int(-(-loads.max() // P)))  # min gather chunks per dst tile

    row_of = tile_of * P + pos_of  # global new row of each node

    # --- per-edge placement (non-self edges) --------------------------
    e_tile = tile_of[dst]
    e_dslot = pos_of[dst].astype(np.int64)
    e_srcrow = row_of[src]
    e_val = dinv[dst]  # S nonzero value = dinv of the destination

    sort_idx = np.lexsort((e_srcrow, e_tile))
    e_tile = e_tile[sort_idx]
    e_dslot = e_dslot[sort_idx]
    e_srcrow = e_srcrow[sort_idx]
    e_val = e_val[sort_idx]
    nE = len(e_tile)

    # --- window split (dma_gather int16 limit) ------------------------
    WA = min(WINDOW_CAP, NG)  # window A = rows [0, WA)
    WB_off = max(NG - WINDOW_CAP, 0)  # window B = rows [WB_off, NG)
    use_B = WB_off > 0

    tile_n = np.bincount(e_tile, minlength=n_tiles)
    if use_B:
        mustA = e_srcrow < WB_off
        mustB = e_srcrow >= WA
        flex = ~mustA & ~mustB
        cntA = np.bincount(e_tile[mustA], minlength=n_tiles)
        cntB = np.bincount(e_tile[mustB], minlength=n_tiles)
        # find (K_A, K_B) with K_A+K_B minimal and all tiles feasible
        found = None
        K_tot = K
        while found is None:
            mid = -(-K_tot // 2)
            for d in range(K_tot + 1):
                for K_A in {mid + d, mid - d}:
                    if not 0 <= K_A <= K_tot:
                        continue
                    K_B = K_tot - K_A
                    if (
                        cntA.max() <= K_A * P
                        and cntB.max() <= K_B * P
                        and tile_n.max() <= (K_A + K_B) * P
                    ):
                        found = (K_A, K_B)
                        break
                if found:
                    break
            if not found:
                K_tot += 1
        K_A, K_B = found
        capB = K_B * P
        # how many of each tile's flex edges go to window A
        nA_t = np.minimum(K_A * P, cntA + np.bincount(
            e_tile[flex], minlength=n_tiles))
        nA_t = np.maximum(nA_t, tile_n - capB)
        flexA_quota = nA_t - cntA
        flex_idx = np.flatnonzero(flex)
        ft = e_tile[flex_idx]
        fstart = np.zeros(n_tiles + 1, dtype=np.int64)
        np.cumsum(np.bincount(ft, minlength=n_tiles), out=fstart[1:])
        frank = np.arange(len(ft)) - fstart[ft]
        toA = mustA.copy()
        toA[flex_idx[frank < flexA_quota[ft]]] = True
    else:
        K_A, K_B = K, 0
        toA = np.ones(nE, dtype=bool)
    K_tot = K_A + K_B
    KC = K_tot + 1  # chunk columns per tile incl. the self chunk

    # --- chunk/slot assignment within each (tile, window) -------------
    e_j = np.empty(nE, dtype=np.int64)  # position within its window list
    e_idxval = np.empty(nE, dtype=np.int64)  # int16 index value
    for is_A in (True, False):
        m = toA if is_A else ~toA
        if not m.any():
            continue
        idxs = np.flatnonzero(m)
        t_sel = e_tile[idxs]
        start = np.zeros(n_tiles + 1, dtype=np.int64)
        np.cumsum(np.bincount(t_sel, minlength=n_tiles), out=start[1:])
        e_j[idxs] = np.arange(len(idxs)) - start[t_sel]
        e_idxval[idxs] = e_srcrow[idxs] - (0 if is_A else WB_off)

    e_kloc = e_j // P  # chunk within window
    e_p = e_j % P
    e_chunk = np.where(toA, e_kloc, K_A + e_kloc)  # chunk within tile

    e_core = e_tile // T
    e_t_in_core = e_tile % T

    # idx table: per gather block of 8*K_w columns; value j at
    # [j%16, j//16], replicated across the 8 groups of 16 partitions.
    idx_cols = T * K_tot * 8
    idx16 = np.zeros((N_CORES, 16, idx_cols), dtype=np.int16)
    AOFF = T * K_A * 8
    blk_base = np.where(toA, e_t_in_core * K_A * 8,
                        AOFF + e_t_in_core * K_B * 8)
    idx16[e_core, e_j % 16, blk_base + e_j // 16] = e_idxval.astype(np.int16)
    idxT = np.tile(idx16, (1, P // 16, 1))

    # dense S (bf16 on the wire): S[p, col*P + d] = norm of the edge
    S = np.zeros((N_CORES, P, T * KC * P), dtype=np.float32)
    col = e_t_in_core * KC + e_chunk
    S[e_core, e_p, col * P + e_dslot] = e_val

    # self chunk (k == K_tot): diagonal dinv
    n_core = (tile_of // T).astype(np.int64)
    n_t_in_core = tile_of % T
    n_slot = pos_of
    scol = n_t_in_core * KC + K_tot
    S[n_core, n_slot, scol * P + n_slot] = dinv

    # per-node dinv (for scaling GEMM outputs); 0 for empty slots
    dinvn = np.zeros((N_CORES, P, T), dtype=np.float32)
    dinvn[n_core, n_slot, n_t_in_core] = dinv

    # --- per-core transposed, tile-blocked node features --------------
    KI = -(-IN // P)
    IN_pad = KI * P
    xf = np.zeros((NG, IN_pad), dtype=np.float32)
    xf[row_of, :IN] = x
    # xt[c, p_in, (t*KI+ki)*P + n] = x[node(c,t,n), ki*P + p_in]
    xt = (
        xf.reshape(N_CORES, T, P, KI, P)
        .transpose(0, 4, 1, 3, 2)
        .reshape(N_CORES, P, T * KI * P)
    )

    meta = dict(
        N=N, IN=IN, IN_pad=IN_pad, T=T, K_A=K_A, K_B=K_B, K=K_tot,
        NPC=NPC, NG=NG, WA=WA, WB_off=WB_off,
        node_core=n_core, node_col=n_t_in_core * P + n_slot,
    )
    return xt, idxT, S, dinvn, meta


def _assemble(outs, meta, OUT):
    """Gather per-core outputs back to the original node order."""
    N = meta["N"]
    full = np.empty((N, OUT), dtype=np.float32)
    node_core = meta["node_core"]
    node_col = meta["node_col"]
    for c in range(N_CORES):
        m = node_core == c
        full[m] = outs[c][node_col[m]]
    return full


# -------------------------------------------------------------- device side


def _build_program(T, K_A, K_B, KI, HID, OUT, NPC, NG, WA, WB_off, n_cores):
    import concourse.bacc as bacc
    import concourse.tile as tile
    from concourse import mybir

    f32 = mybir.dt.float32
    bf16 = mybir.dt.bfloat16
    i16 = mybir.dt.int16
    K = K_A + K_B
    KC = K + 1
    KH = HID // P  # 128-chunks of hidden dim
    Relu = mybir.ActivationFunctionType.Relu
    Copy = mybir.ActivationFunctionType.Copy
    EQ = mybir.AluOpType.is_equal
    MUL = mybir.AluOpType.mult

    nc = bacc.Bacc(
        "TRN2", target_bir_lowering=False, debug=False, num_devices=n_cores,
        num_swdge_queues=4,
    )

    xt = nc.dram_tensor("xt", [P, T * KI * P], bf16, kind="ExternalInput").ap()
    w1 = nc.dram_tensor("w1", [P, KI * HID], bf16, kind="ExternalInput").ap()
    b1 = nc.dram_tensor("b1", [1, HID], bf16, kind="ExternalInput").ap()
    w2 = nc.dram_tensor("w2", [P, KH * OUT], bf16, kind="ExternalInput").ap()
    b2 = nc.dram_tensor("b2", [1, OUT], bf16, kind="ExternalInput").ap()
    s_in = nc.dram_tensor("s", [P, T * KC * P], bf16, kind="ExternalInput").ap()
    dinvn = nc.dram_tensor("dinvn", [P, T], f32, kind="ExternalInput").ap()
    idxt = nc.dram_tensor("idxt", [P, T * K * 8], i16, kind="ExternalInput").ap()
    out = nc.dram_tensor("out", [NPC, OUT], f32, kind="ExternalOutput").ap()

    rg = [list(range(n_cores))]
    qn = [0]

    def next_q():
        q = qn[0]
        qn[0] = (q + 1) % 4
        return q

    with tile.TileContext(nc) as tc:
        with (
            tc.tile_pool(name="dram", bufs=1, space="DRAM") as dpool,
            tc.tile_pool(name="const", bufs=1) as cpool,
            tc.tile_pool(name="work", bufs=3) as wpool,
            tc.tile_pool(name="gath", bufs=2) as gpool,
            tc.tile_pool(name="sblk", bufs=2) as spool,
            tc.tile_pool(name="pers", bufs=1) as ppool,
            tc.tile_pool(name="ps", bufs=2, space="PSUM") as pspool,
        ):
            h1_loc = dpool.tile([NPC, HID], bf16)
            h1_full = dpool.tile([NG, HID], bf16, addr_space="Shared")
            h2_loc = dpool.tile([NPC, OUT], bf16)
            h2_full = dpool.tile([NG, OUT], bf16, addr_space="Shared")

            # ---- constants -----------------------------------------------
            w1_sb = cpool.tile([P, KI * HID], bf16)
            nc.sync.dma_start(out=w1_sb[:], in_=w1[:])
            w2_sb = cpool.tile([P, KH * OUT], bf16)
            nc.sync.dma_start(out=w2_sb[:], in_=w2[:])
            b1_sb = cpool.tile([1, HID], bf16)
            nc.sync.dma_start(out=b1_sb[:], in_=b1[:])
            b2_sb = cpool.tile([1, OUT], bf16)
            nc.sync.dma_start(out=b2_sb[:], in_=b2[:])
            ones1 = cpool.tile([1, P], bf16)
            nc.gpsimd.memset(ones1[:], 1.0)
            dinvn_sb = cpool.tile([P, T], f32)
            nc.sync.dma_start(out=dinvn_sb[:], in_=dinvn[:])
            idx_sb = cpool.tile([P, T * K * 8], i16)
            nc.sync.dma_start(out=idx_sb[:], in_=idxt[:])

            a1T = ppool.tile([P, KH * NPC], bf16)  # transposed activations

            def load_s(pool_tag, t):
                s_sb = spool.tile([P, KC * P], bf16, tag=pool_tag, name="s_sb")
                nc.sync.dma_start(
                    out=s_sb[:], in_=s_in[:, t * KC * P:(t + 1) * KC * P]
                )
                return s_sb

            GRP = 4  # tiles per gather call
            AOFF = T * K_A * 8

            def gathers(t0, geff, h_full, h_loc, F, tag):
                """Grouped windowed dma_gathers + per-tile self DMAs for
                tiles [t0, t0+geff); returns chunk(j, k) -> [128, F]."""
                gA = gpool.tile([P, GRP * max(K_A, 1) * F], bf16,
                                tag=tag + "A")
                if K_A > 0:
                    n = geff * K_A * P
                    nc.gpsimd.dma_gather(
                        out_ap=gA[:, 0:geff * K_A * F].rearrange(
                            "p (k e) -> p k e", e=F),
                        in_ap=h_full[0:WA, :],
                        idxs_ap=idx_sb[:, t0 * K_A * 8:
                                       (t0 + geff) * K_A * 8],
                        num_idxs=n,
                        num_idxs_reg=n,
                        elem_size=F,
                        single_packet=False,
                        queue_num=next_q(),
                    )
                gB = None
                if K_B > 0:
                    gB = gpool.tile([P, GRP * K_B * F], bf16, tag=tag + "B")
                    n = geff * K_B * P
                    nc.gpsimd.dma_gather(
                        out_ap=gB[:, 0:geff * K_B * F].rearrange(
                            "p (k e) -> p k e", e=F),
                        in_ap=h_full[WB_off:NG, :],
                        idxs_ap=idx_sb[:, AOFF + t0 * K_B * 8:
                                       AOFF + (t0 + geff) * K_B * 8],
                        num_idxs=n,
                        num_idxs_reg=n,
                        elem_size=F,
                        single_packet=False,
                        queue_num=next_q(),
                    )
                gSs = []
                for j in range(geff):
                    t = t0 + j
                    gS = gpool.tile([P, F], bf16, tag=tag + "S",
                                    name="gS", bufs=8)
                    nc.sync.dma_start(
                        out=gS[:], in_=h_loc[t * P:(t + 1) * P, :])
                    gSs.append(gS)

                def chunk(j, k):
                    if k < K_A:
                        return gA[:, (j * K_A + k) * F:
                                  (j * K_A + k + 1) * F]
                    if k < K:
                        kb = j * K_B + (k - K_A)
                        return gB[:, kb * F:(kb + 1) * F]
                    return gSs[j][:]

                return chunk

            # ---- layer-1 GEMM: h1 = dinv * (x @ W1), AllGather ----------
            for t in range(T):
                xt_t = wpool.tile([P, KI * P], bf16, tag="xt")
                nc.sync.dma_start(
                    out=xt_t[:], in_=xt[:, t * KI * P:(t + 1) * KI * P]
                )
                ps_h = pspool.tile([P, HID], f32, tag="ps_h")
                for ki in range(KI):
                    nc.tensor.matmul(
                        ps_h[:],
                        lhsT=xt_t[:, ki * P:(ki + 1) * P],
                        rhs=w1_sb[:, ki * HID:(ki + 1) * HID],
                        start=(ki == 0),
                        stop=(ki == KI - 1),
                    )
                h1t = wpool.tile([P, HID], bf16, tag="h1t")
                nc.vector.tensor_scalar_mul(
                    h1t[:], ps_h[:], dinvn_sb[:, t:t + 1]
                )
                nc.sync.dma_start(
                    out=h1_loc[t * P:(t + 1) * P, :], in_=h1t[:]
                )

            nc.gpsimd.collective_compute(
                "AllGather",
                mybir.AluOpType.bypass,
                replica_groups=rg,
                ins=[h1_loc.opt()],
                outs=[h1_full.opt()],
            )

            # ---- layer-1 aggregation (transposed) + relu + layer-2 GEMM --
            for t0 in range(0, T, 4):
              geff = min(4, T - t0)
              chunk_g = gathers(t0, geff, h1_full, h1_loc, HID, "g1")
              for j in range(geff):
                t = t0 + j
                chunk = lambda k, _j=j: chunk_g(_j, k)
                psa_t = pspool.tile([P, KH * P], f32, tag="ps_a", bufs=3)
                psa = [psa_t[:, fh * P:(fh + 1) * P] for fh in range(KH)]
                for fh in range(KH):
                    nc.tensor.matmul(
                        psa[fh][:],
                        lhsT=b1_sb[0:1, fh * P:(fh + 1) * P],
                        rhs=ones1[0:1, :],
                        start=True,
                        stop=False,
                    )
                s_sb = load_s("s1", t)
                for k in range(KC):
                    g = chunk(k)
                    for fh in range(KH):
                        nc.tensor.matmul(
                            psa[fh][:],
                            lhsT=g[:, fh * P:(fh + 1) * P],
                            rhs=s_sb[:, k * P:(k + 1) * P],
                            start=False,
                            stop=(k == KC - 1),
                        )
                for fh in range(KH):
                    nc.scalar.activation(
                        out=a1T[:, fh * NPC + t * P: fh * NPC + (t + 1) * P],
                        in_=psa[fh][:],
                        func=Relu,
                    )
                # layer-2 GEMM for this tile: h2 = dinv * (a1 @ W2)
                ps2 = pspool.tile([P, OUT], f32, tag="ps_o")
                for kh in range(KH):
                    nc.tensor.matmul(
                        ps2[:],
                        lhsT=a1T[:, kh * NPC + t * P: kh * NPC + (t + 1) * P],
                        rhs=w2_sb[:, kh * OUT:(kh + 1) * OUT],
                        start=(kh == 0),
                        stop=(kh == KH - 1),
                    )
                h2t = wpool.tile([P, OUT], bf16, tag="h2t")
                nc.vector.tensor_scalar_mul(
                    h2t[:], ps2[:], dinvn_sb[:, t:t + 1]
                )
                nc.sync.dma_start(
                    out=h2_loc[t * P:(t + 1) * P, :], in_=h2t[:]
                )

            nc.gpsimd.collective_compute(
                "AllGather",
                mybir.AluOpType.bypass,
                replica_groups=rg,
                ins=[h2_loc.opt()],
                outs=[h2_full.opt()],
            )

            # ---- layer-2 aggregation: out = S^T @ h2[idx] + b2 -----------
            for t0 in range(0, T, 4):
              geff = min(4, T - t0)
              chunk_g = gathers(t0, geff, h2_full, h2_loc, OUT, "g2")
              for j in range(geff):
                t = t0 + j
                chunk = lambda k, _j=j: chunk_g(_j, k)
                pso = pspool.tile([P, OUT], f32, tag="ps_o")
                nc.tensor.matmul(
                    pso[:], lhsT=ones1[0:1, :], rhs=b2_sb[0:1, :],
                    start=True, stop=False,
                )
                s_sb = load_s("s2", t)
                for k in range(KC):
                    nc.tensor.matmul(
                        pso[:],
                        lhsT=s_sb[:, k * P:(k + 1) * P],
                        rhs=chunk(k),
                        start=False,
                        stop=(k == KC - 1),
                    )
                ot = wpool.tile([P, OUT], f32, tag="ot")
                nc.scalar.activation(out=ot[:], in_=pso[:], func=Copy)
                nc.sync.dma_start(out=out[t * P:(t + 1) * P, :], in_=ot[:])

    nc.compile()
    return nc


def _get_program(T, K_A, K_B, KI, HID, OUT, NPC, NG, WA, WB_off,
                 n_cores=N_CORES):
    key = (T, K_A, K_B, KI, HID, OUT, NPC, NG, WA, WB_off, n_cores)
    if key not in _prog_cache:
        _prog_cache[key] = _build_program(
            T, K_A, K_B, KI, HID, OUT, NPC, NG, WA, WB_off, n_cores
        )
    return _prog_cache[key]


# ------------------------------------------------------------------- driver


def _make_in_maps(x, edge_index, W1, b1, W2, b2):
    W1 = np.ascontiguousarray(np.asarray(W1, dtype=np.float32))
    W2 = np.ascontiguousarray(np.asarray(W2, dtype=np.float32))
    b1 = np.ascontiguousarray(np.asarray(b1, dtype=np.float32)).reshape(1, -1)
    b2 = np.ascontiguousarray(np.asarray(b2, dtype=np.float32)).reshape(1, -1)
    xt, idxT, S, dinvn, meta = _preprocess(x, edge_index)
    IN_pad = meta["IN_pad"]
    KI = IN_pad // P
    HID = W1.shape[1]
    OUT = W2.shape[1]
    if W1.shape[0] < IN_pad:
        W1 = np.concatenate(
            [W1, np.zeros((IN_pad - W1.shape[0], HID), np.float32)], axis=0
        )
    # w1 tiled: [P, KI*HID], block ki = W1[ki*P:(ki+1)*P, :]
    w1t = W1.reshape(KI, P, HID).transpose(1, 0, 2).reshape(P, KI * HID)
    KH = HID // P
    w2t = W2.reshape(KH, P, OUT).transpose(1, 0, 2).reshape(P, KH * OUT)

    in_maps = [
        {
            "xt": _bf16(xt[c]),
            "w1": _bf16(w1t),
            "b1": _bf16(b1),
            "w2": _bf16(w2t),
            "b2": _bf16(b2),
            "s": _bf16(S[c]),
            "dinvn": dinvn[c],
            "idxt": idxT[c],
        }
        for c in range(N_CORES)
    ]
    return in_maps, meta, HID, OUT


def run(x, edge_index, W1, b1, W2, b2, trace=False, trace_cores=None):
    from concourse.bass_utils import run_bass_kernel_spmd

    in_maps, meta, HID, OUT = _make_in_maps(x, edge_index, W1, b1, W2, b2)
    nc = _get_program(
        meta["T"], meta["K_A"], meta["K_B"], meta["IN_pad"] // P, HID, OUT,
        meta["NPC"], meta["NG"], meta["WA"], meta["WB_off"],
    )
    res = run_bass_kernel_spmd(
        nc,
        in_maps,
        core_ids=list(range(N_CORES)),
        trace=trace,
        trace_cores=trace_cores,
    )
    outs = [res.results[c]["out"] for c in range(N_CORES)]
    return _assemble(outs, meta, OUT), res


def kernel(x, edge_index, W1, b1, W2, b2):
    full, _ = run(x, edge_index, W1, b1, W2, b2, trace=False)
    return full


# revision 15
# speedup vs baseline: 1.1029x; 1.1029x over previous
"""Two-layer GCN (PyG GCNConv-style) on 8 Trainium2 NeuronCores.

Strategy: nodes are partitioned across the 8 cores (load-balanced into
128-row tiles by in-degree), edges partitioned by destination node so
the segment-sum is local.  Each layer is transform-first: local GEMM
(h = x @ W, scaled by dinv[src]), AllGather of the bf16 transformed
features, then a local gather + segment-sum over incoming edges.

The segment-sum runs on the TensorEngine: for each destination tile of
128 nodes, its incoming edges (chunked by 128) are bulk-gathered with
dma_gather into SBUF [128edges x F] per chunk (bf16, halving DMA), and
contracted with a one-hot matrix S [128edges x 128dst] whose nonzeros
carry dinv[dst].  S is host-built and streamed as dense bf16 (the DMA
engines have slack; building S on the vector engine was measured to be
the bottleneck).  The gathers are spread round-robin over 4 SWDGE
queues (num_swdge_queues=4), which parallelizes the per-index Q7
descriptor-generation ucode (~8.7ns/idx serial, measured) about 4x -
this, not bytes, is the dominant cost of dma_gather.

Layer-1 aggregation is computed TRANSPOSED (psxT[f,d] = sum_k G_k^T @
S_k) so the relu output lands directly in lhsT layout for the layer-2
GEMM - no on-device transposes at all.  norm = dinv[src]*dinv[dst] is
factored: dinv[src] is folded into the AllGathered features (free, at
the GEMM psum->sbuf copy), dinv[dst] into the S nonzeros.  Self-loops
use a plain contiguous DMA of the local tile plus a diagonal S block
built by the same machinery (dst=iota, val=dinv).

dma_gather takes int16 row indices, so the gathered table is addressed
through two overlapping <=32767-row windows.
"""

import numpy as np

P = 128
N_CORES = 8
WINDOW_CAP = 32512  # dma_gather int16 window (multiple of 128, <= 32767)

_prog_cache = {}


# ---------------------------------------------------------------- host side


def _bf16(a):
    import ml_dtypes

    return np.asarray(a, dtype=ml_dtypes.bfloat16)


def _preprocess(x, edge_index):
    """Partition nodes/edges, build per-core device arrays."""
    x = np.ascontiguousarray(np.asarray(x, dtype=np.float32))
    ei = np.asarray(edge_index)
    N, IN = x.shape

    src = ei[0].astype(np.int64)
    dst = ei[1].astype(np.int64)

    deg = 1 + np.bincount(dst, minlength=N)  # with self loop, >= 1
    dinv = (1.0 / np.sqrt(deg.astype(np.float64))).astype(np.float32)

    npc_nodes = -(-N // N_CORES)
    T = -(-npc_nodes // P)  # dst tiles per core
    NPC = T * P  # node slots per core
    n_tiles = N_CORES * T
    NG = n_tiles * P  # global node slots

    # --- pack nodes into tiles, balancing per-tile in-degree (LPT) ----
    import heapq

    degg = deg - 1  # gathered (non-self) in-degree
    tile_of = np.empty(N, dtype=np.int64)
    pos_of = np.empty(N, dtype=np.int64)
    counts = np.zeros(n_tiles, dtype=np.int64)
    loads = np.zeros(n_tiles, dtype=np.int64)
    order = np.argsort(-degg, kind="stable")
    heap = [(0, t) for t in range(n_tiles)]
    heapq.heapify(heap)
    deg_l = degg[order]
    for i in range(N):
        v = order[i]
        while True:
            load, t = heapq.heappop(heap)
            if counts[t] < P:
                break
        tile_of[v] = t
        pos_of[v] = counts[t]
        counts[t] += 1
        load += int(deg_l[i])
        loads[t] = load
        if counts[t] < P:
            heapq.heappush(heap, (load, t))

    # repair pass: move small nodes off overloaded tiles to reach the
    # ideal chunk count ceil(total/(n_tiles*P)) if possible
    K_ideal = max(1, int(-(-int(degg.sum()) // (n_tiles * P))))
    target = K_ideal * P
    if loads.max() > target:
        by_tile = [[] for _ in range(n_tiles)]
        for i in range(N - 1, -1, -1):  # ascending degree order
            by_tile[tile_of[order[i]]].append(order[i])
        free = [(loads[t], t) for t in range(n_tiles)
                if counts[t] < P and loads[t] < target]
        heapq.heapify(free)
        for t_over in np.flatnonzero(loads > target):
            stack = by_tile[t_over]
            si = 0
            while loads[t_over] > target and si < len(stack) and free:
                v = stack[si]
                si += 1
                d = int(degg[v])
                moved = False
                tried = []
                while free:
                    lo, t2 = heapq.heappop(free)
                    if lo != loads[t2] or counts[t2] >= P:
                        continue  # stale
                    if loads[t2] + d <= target:
                        tile_of[v] = t2
                        pos_of[v] = counts[t2]
                        counts[t2] += 1
                        loads[t2] += d
                        loads[t_over] -= d
                        moved = True
                        if counts[t2] < P and loads[t2] < target:
                            heapq.heappush(free, (loads[t2], t2))
                        break
                    tried.append((lo, t2))
                for it in tried:
                    heapq.heappush(free, it)
                if not moved:
                    break
        # recompute pos_of consistently (holes possible after moves)
        ordv = np.lexsort((np.arange(N), tile_of))
        pos = np.empty(N, dtype=np.int64)
        tt = tile_of[ordv]
        st = np.zeros(n_tiles + 1, dtype=np.int64)
        np.cumsum(np.bincount(tt, minlength=n_tiles), out=st[1:])
        pos[ordv] = np.arange(N) - st[tt]
        pos_of = pos

    K = max(1, int(-(-loads.max() // P)))  # min gather chunks per dst tile

    row_of = tile_of * P + pos_of  # global new row of each node

    # --- per-edge placement (non-self edges) --------------------------
    e_tile = tile_of[dst]
    e_dslot = pos_of[dst].astype(np.int64)
    e_srcrow = row_of[src]
    e_val = dinv[dst]  # S nonzero value = dinv of the destination

    sort_idx = np.lexsort((e_srcrow, e_tile))
    e_tile = e_tile[sort_idx]
    e_dslot = e_dslot[sort_idx]
    e_srcrow = e_srcrow[sort_idx]
    e_val = e_val[sort_idx]
    nE = len(e_tile)

    # --- window split (dma_gather int16 limit) ------------------------
    WA = min(WINDOW_CAP, NG)  # window A = rows [0, WA)
    WB_off = max(NG - WINDOW_CAP, 0)  # window B = rows [WB_off, NG)
    use_B = WB_off > 0

    tile_n = np.bincount(e_tile, minlength=n_tiles)
    if use_B:
        mustA = e_srcrow < WB_off
        mustB = e_srcrow >= WA
        flex = ~mustA & ~mustB
        cntA = np.bincount(e_tile[mustA], minlength=n_tiles)
        cntB = np.bincount(e_tile[mustB], minlength=n_tiles)
        # find (K_A, K_B) with K_A+K_B minimal and all tiles feasible
        found = None
        K_tot = K
        while found is None:
            mid = -(-K_tot // 2)
            for d in range(K_tot + 1):
                for K_A in {mid + d, mid - d}:
                    if not 0 <= K_A <= K_tot:
                        continue
                    K_B = K_tot - K_A
                    if (
                        cntA.max() <= K_A * P
                        and cntB.max() <= K_B * P
                        and tile_n.max() <= (K_A + K_B) * P
                    ):
                        found = (K_A, K_B)
                        break
                if found:
                    break
            if not found:
                K_tot += 1
        K_A, K_B = found
        capB = K_B * P
        # how many of each tile's flex edges go to window A
        nA_t = np.minimum(K_A * P, cntA + np.bincount(
            e_tile[flex], minlength=n_tiles))
        nA_t = np.maximum(nA_t, tile_n - capB)
        flexA_quota = nA_t - cntA
        flex_idx = np.flatnonzero(flex)
        ft = e_tile[flex_idx]
        fstart = np.zeros(n_tiles + 1, dtype=np.int64)
        np.cumsum(np.bincount(ft, minlength=n_tiles), out=fstart[1:])
        frank = np.arange(len(ft)) - fstart[ft]
        toA = mustA.copy()
        toA[flex_idx[frank < flexA_quota[ft]]] = True
    else:
        K_A, K_B = K, 0
        toA = np.ones(nE, dtype=bool)
    K_tot = K_A + K_B
    KC = K_tot + 1  # chunk columns per tile incl. the self chunk

    # --- chunk/slot assignment within each (tile, window) -------------
    e_j = np.empty(nE, dtype=np.int64)  # position within its window list
    e_idxval = np.empty(nE, dtype=np.int64)  # int16 index value
    for is_A in (True, False):
        m = toA if is_A else ~toA
        if not m.any():
            continue
        idxs = np.flatnonzero(m)
        t_sel = e_tile[idxs]
        start = np.zeros(n_tiles + 1, dtype=np.int64)
        np.cumsum(np.bincount(t_sel, minlength=n_tiles), out=start[1:])
        e_j[idxs] = np.arange(len(idxs)) - start[t_sel]
        e_idxval[idxs] = e_srcrow[idxs] - (0 if is_A else WB_off)

    e_kloc = e_j // P  # chunk within window
    e_p = e_j % P
    e_chunk = np.where(toA, e_kloc, K_A + e_kloc)  # chunk within tile

    e_core = e_tile // T
    e_t_in_core = e_tile % T

    # idx table: per gather block of 8*K_w columns; value j at
    # [j%16, j//16], replicated across the 8 groups of 16 partitions.
    idx_cols = T * K_tot * 8
    idx16 = np.zeros((N_CORES, 16, idx_cols), dtype=np.int16)
    blk_base = e_t_in_core * K_tot * 8 + np.where(toA, 0, K_A * 8)
    idx16[e_core, e_j % 16, blk_base + e_j // 16] = e_idxval.astype(np.int16)
    idxT = np.tile(idx16, (1, P // 16, 1))

    # dense S (bf16 on the wire): S[p, col*P + d] = norm of the edge
    S = np.zeros((N_CORES, P, T * KC * P), dtype=np.float32)
    col = e_t_in_core * KC + e_chunk
    S[e_core, e_p, col * P + e_dslot] = e_val

    # self chunk (k == K_tot): diagonal dinv
    n_core = (tile_of // T).astype(np.int64)
    n_t_in_core = tile_of % T
    n_slot = pos_of
    scol = n_t_in_core * KC + K_tot
    S[n_core, n_slot, scol * P + n_slot] = dinv

    # per-node dinv (for scaling GEMM outputs); 0 for empty slots
    dinvn = np.zeros((N_CORES, P, T), dtype=np.float32)
    dinvn[n_core, n_slot, n_t_in_core] = dinv

    # --- per-core transposed, tile-blocked node features --------------
    KI = -(-IN // P)
    IN_pad = KI * P
    xf = np.zeros((NG, IN_pad), dtype=np.float32)
    xf[row_of, :IN] = x
    # xt[c, p_in, (t*KI+ki)*P + n] = x[node(c,t,n), ki*P + p_in]
    xt = (
        xf.reshape(N_CORES, T, P, KI, P)
        .transpose(0, 4, 1, 3, 2)
        .reshape(N_CORES, P, T * KI * P)
    )

    meta = dict(
        N=N, IN=IN, IN_pad=IN_pad, T=T, K_A=K_A, K_B=K_B, K=K_tot,
        NPC=NPC, NG=NG, WA=WA, WB_off=WB_off,
        node_core=n_core, node_col=n_t_in_core * P + n_slot,
    )
    return xt, idxT, S, dinvn, meta


def _assemble(outs, meta, OUT):
    """Gather per-core outputs back to the original node order."""
    N = meta["N"]
    full = np.empty((N, OUT), dtype=np.float32)
    node_core = meta["node_core"]
    node_col = meta["node_col"]
    for c in range(N_CORES):
        m = node_core == c
        full[m] = outs[c][node_col[m]]
    return full


# -------------------------------------------------------------- device side


def _build_program(T, K_A, K_B, KI, HID, OUT, NPC, NG, WA, WB_off, n_cores):
    import concourse.bacc as bacc
    import concourse.tile as tile
    from concourse import mybir

    f32 = mybir.dt.float32
    bf16 = mybir.dt.bfloat16
    i16 = mybir.dt.int16
    K = K_A + K_B
    KC = K + 1
    KH = HID // P  # 128-chunks of hidden dim
    Relu = mybir.ActivationFunctionType.Relu
    Copy = mybir.ActivationFunctionType.Copy
    EQ = mybir.AluOpType.is_equal
    MUL = mybir.AluOpType.mult

    nc = bacc.Bacc(
        "TRN2", target_bir_lowering=False, debug=False, num_devices=n_cores,
        num_swdge_queues=4,
    )

    xt = nc.dram_tensor("xt", [P, T * KI * P], bf16, kind="ExternalInput").ap()
    w1 = nc.dram_tensor("w1", [P, KI * HID], bf16, kind="ExternalInput").ap()
    b1 = nc.dram_tensor("b1", [1, HID], bf16, kind="ExternalInput").ap()
    w2 = nc.dram_tensor("w2", [P, KH * OUT], bf16, kind="ExternalInput").ap()
    b2 = nc.dram_tensor("b2", [1, OUT], bf16, kind="ExternalInput").ap()
    s_in = nc.dram_tensor("s", [P, T * KC * P], bf16, kind="ExternalInput").ap()
    dinvn = nc.dram_tensor("dinvn", [P, T], f32, kind="ExternalInput").ap()
    idxt = nc.dram_tensor("idxt", [P, T * K * 8], i16, kind="ExternalInput").ap()
    out = nc.dram_tensor("out", [NPC, OUT], f32, kind="ExternalOutput").ap()

    rg = [list(range(n_cores))]
    qn = [0]

    def next_q():
        q = qn[0]
        qn[0] = (q + 1) % 4
        return q

    with tile.TileContext(nc) as tc:
        with (
            tc.tile_pool(name="dram", bufs=1, space="DRAM") as dpool,
            tc.tile_pool(name="const", bufs=1) as cpool,
            tc.tile_pool(name="work", bufs=3) as wpool,
            tc.tile_pool(name="gath", bufs=2) as gpool,
            tc.tile_pool(name="sblk", bufs=2) as spool,
            tc.tile_pool(name="pers", bufs=1) as ppool,
            tc.tile_pool(name="ps", bufs=2, space="PSUM") as pspool,
        ):
            h1_loc = dpool.tile([NPC, HID], bf16)
            h1_full = dpool.tile([NG, HID], bf16, addr_space="Shared")
            h2_loc = dpool.tile([NPC, OUT], bf16)
            h2_full = dpool.tile([NG, OUT], bf16, addr_space="Shared")

            # ---- constants -----------------------------------------------
            w1_sb = cpool.tile([P, KI * HID], bf16)
            nc.sync.dma_start(out=w1_sb[:], in_=w1[:])
            w2_sb = cpool.tile([P, KH * OUT], bf16)
            nc.sync.dma_start(out=w2_sb[:], in_=w2[:])
            b1_sb = cpool.tile([1, HID], bf16)
            nc.sync.dma_start(out=b1_sb[:], in_=b1[:])
            b2_sb = cpool.tile([1, OUT], bf16)
            nc.sync.dma_start(out=b2_sb[:], in_=b2[:])
            ones1 = cpool.tile([1, P], bf16)
            nc.gpsimd.memset(ones1[:], 1.0)
            dinvn_sb = cpool.tile([P, T], f32)
            nc.sync.dma_start(out=dinvn_sb[:], in_=dinvn[:])
            idx_sb = cpool.tile([P, T * K * 8], i16)
            nc.sync.dma_start(out=idx_sb[:], in_=idxt[:])

            a1T = ppool.tile([P, KH * NPC], bf16)  # transposed activations

            def load_s(pool_tag, t):
                s_sb = spool.tile([P, KC * P], bf16, tag=pool_tag,
                                  name="s_sb", bufs=8)
                nc.sync.dma_start(
                    out=s_sb[:], in_=s_in[:, t * KC * P:(t + 1) * KC * P]
                )
                return s_sb

            def gathers(t, h_full, h_loc, F, tag):
                """Windowed dma_gathers + self-chunk DMA for dst tile t;
                returns chunk k -> gathered [128, F] slice (k == K: self)."""
                blk = t * K * 8
                gA = gpool.tile([P, max(K_A, 1) * F], bf16, tag=tag + "A")
                if K_A > 0:
                    nc.gpsimd.dma_gather(
                        out_ap=gA[:].rearrange("p (k e) -> p k e", e=F),
                        in_ap=h_full[0:WA, :],
                        idxs_ap=idx_sb[:, blk:blk + K_A * 8],
                        num_idxs=K_A * P,
                        num_idxs_reg=K_A * P,
                        elem_size=F,
                        single_packet=False,
                        queue_num=next_q(),
                    )
                gB = None
                if K_B > 0:
                    gB = gpool.tile([P, K_B * F], bf16, tag=tag + "B")
                    nc.gpsimd.dma_gather(
                        out_ap=gB[:].rearrange("p (k e) -> p k e", e=F),
                        in_ap=h_full[WB_off:NG, :],
                        idxs_ap=idx_sb[:, blk + K_A * 8:blk + K * 8],
                        num_idxs=K_B * P,
                        num_idxs_reg=K_B * P,
                        elem_size=F,
                        single_packet=False,
                        queue_num=next_q(),
                    )
                gS = gpool.tile([P, F], bf16, tag=tag + "S", bufs=4)
                nc.sync.dma_start(out=gS[:], in_=h_loc[t * P:(t + 1) * P, :])

                def chunk(k):
                    if k < K_A:
                        return gA[:, k * F:(k + 1) * F]
                    if k < K:
                        return gB[:, (k - K_A) * F:(k - K_A + 1) * F]
                    return gS[:]

                return chunk

            # ---- layer-1 GEMM: h1 = dinv * (x @ W1), AllGather ----------
            for t in range(T):
                xt_t = wpool.tile([P, KI * P], bf16, tag="xt")
                nc.sync.dma_start(
                    out=xt_t[:], in_=xt[:, t * KI * P:(t + 1) * KI * P]
                )
                ps_h = pspool.tile([P, HID], f32, tag="ps_h")
                for ki in range(KI):
                    nc.tensor.matmul(
                        ps_h[:],
                        lhsT=xt_t[:, ki * P:(ki + 1) * P],
                        rhs=w1_sb[:, ki * HID:(ki + 1) * HID],
                        start=(ki == 0),
                        stop=(ki == KI - 1),
                    )
                h1t = wpool.tile([P, HID], bf16, tag="h1t")
                nc.vector.tensor_scalar_mul(
                    h1t[:], ps_h[:], dinvn_sb[:, t:t + 1]
                )
                nc.sync.dma_start(
                    out=h1_loc[t * P:(t + 1) * P, :], in_=h1t[:]
                )

            nc.gpsimd.collective_compute(
                "AllGather",
                mybir.AluOpType.bypass,
                replica_groups=rg,
                ins=[h1_loc.opt()],
                outs=[h1_full.opt()],
            )

            # ---- layer-1 aggregation (transposed) + relu + layer-2 GEMM --
            for t in range(T):
                chunk = gathers(t, h1_full, h1_loc, HID, "g1")
                psa_t = pspool.tile([P, KH * P], f32, tag="ps_a", bufs=3)
                psa = [psa_t[:, fh * P:(fh + 1) * P] for fh in range(KH)]
                for fh in range(KH):
                    nc.tensor.matmul(
                        psa[fh][:],
                        lhsT=b1_sb[0:1, fh * P:(fh + 1) * P],
                        rhs=ones1[0:1, :],
                        start=True,
                        stop=False,
                    )
                s_sb = load_s("s1", t)
                for k in range(KC):
                    g = chunk(k)
                    for fh in range(KH):
                        nc.tensor.matmul(
                            psa[fh][:],
                            lhsT=g[:, fh * P:(fh + 1) * P],
                            rhs=s_sb[:, k * P:(k + 1) * P],
                            start=False,
                            stop=(k == KC - 1),
                        )
                for fh in range(KH):
                    nc.scalar.activation(
                        out=a1T[:, fh * NPC + t * P: fh * NPC + (t + 1) * P],
                        in_=psa[fh][:],
                        func=Relu,
                    )
                # layer-2 GEMM for this tile: h2 = dinv * (a1 @ W2)
                ps2 = pspool.tile([P, OUT], f32, tag="ps_o",
                                  bufs=3)
                for kh in range(KH):
                    nc.tensor.matmul(
                        ps2[:],
                        lhsT=a1T[:, kh * NPC + t * P: kh * NPC + (t + 1) * P],
                        rhs=w2_sb[:, kh * OUT:(kh + 1) * OUT],
                        start=(kh == 0),
                        stop=(kh == KH - 1),
                    )
                h2t = wpool.tile([P, OUT], bf16, tag="h2t")
                nc.vector.tensor_scalar_mul(
                    h2t[:], ps2[:], dinvn_sb[:, t:t + 1]
                )
                nc.sync.dma_start(
                    out=h2_loc[t * P:(t + 1) * P, :], in_=h2t[:]
                )

            nc.gpsimd.collective_compute(
                "AllGather",
                mybir.AluOpType.bypass,
                replica_groups=rg,
                ins=[h2_loc.opt()],
                outs=[h2_full.opt()],
            )

            # ---- layer-2 aggregation: out = S^T @ h2[idx] + b2 -----------
            for t in range(T):
                chunk = gathers(t, h2_full, h2_loc, OUT, "g2")
                pso = pspool.tile([P, OUT], f32, tag="ps_o",
                                  bufs=3)
                nc.tensor.matmul(
                    pso[:], lhsT=ones1[0:1, :], rhs=b2_sb[0:1, :],
                    start=True, stop=False,
                )
                s_sb = load_s("s2", t)
                for k in range(KC):
                    nc.tensor.matmul(
                        pso[:],
                        lhsT=s_sb[:, k * P:(k + 1) * P],
                        rhs=chunk(k),
                        start=False,
                        stop=(k == KC - 1),
                    )
                ot = wpool.tile([P, OUT], f32, tag="ot")
                nc.scalar.activation(out=ot[:], in_=pso[:], func=Copy)
                nc.sync.dma_start(out=out[t * P:(t + 1) * P, :], in_=ot[:])

    nc.compile()
    return nc


def _get_program(T, K_A, K_B, KI, HID, OUT, NPC, NG, WA, WB_off,
                 n_cores=N_CORES):
    key = (T, K_A, K_B, KI, HID, OUT, NPC, NG, WA, WB_off, n_cores)
    if key not in _prog_cache:
        _prog_cache[key] = _build_program(
            T, K_A, K_B, KI, HID, OUT, NPC, NG, WA, WB_off, n_cores
        )
    return _prog_cache[key]


# ------------------------------------------------------------------- driver


def _make_in_maps(x, edge_index, W1, b1, W2, b2):
    W1 = np.ascontiguousarray(np.asarray(W1, dtype=np.float32))
    W2 = np.ascontiguousarray(np.asarray(W2, dtype=np.float32))
    b1 = np.ascontiguousarray(np.asarray(b1, dtype=np.float32)).reshape(1, -1)
    b2 = np.ascontiguousarray(np.asarray(b2, dtype=np.float32)).reshape(1, -1)
    xt, idxT, S, dinvn, meta = _preprocess(x, edge_index)
    IN_pad = meta["IN_pad"]
    KI = IN_pad // P
    HID = W1.shape[1]
    OUT = W2.shape[1]
    if W1.shape[0] < IN_pad:
        W1 = np.concatenate(
            [W1, np.zeros((IN_pad - W1.shape[0], HID), np.float32)], axis=0
        )
    # w1 tiled: [P, KI*HID], block ki = W1[ki*P:(ki+1)*P, :]
    w1t = W1.reshape(KI, P, HID).transpose(1, 0, 2).reshape(P, KI * HID)
    KH = HID // P
    w2t = W2.reshape(KH, P, OUT).transpose(1, 0, 2).reshape(P, KH * OUT)

    in_maps = [
        {
            "xt": _bf16(xt[c]),
            "w1": _bf16(w1t),
            "b1": _bf16(b1),
            "w2": _bf16(w2t),
            "b2": _bf16(b2),
            "s": _bf16(S[c]),
            "dinvn": dinvn[c],
            "idxt": idxT[c],
        }
        for c in range(N_CORES)
    ]
    return in_maps, meta, HID, OUT


def run(x, edge_index, W1, b1, W2, b2, trace=False, trace_cores=None):
    from concourse.bass_utils import run_bass_kernel_spmd

    in_maps, meta, HID, OUT = _make_in_maps(x, edge_index, W1, b1, W2, b2)
    nc = _get_program(
        meta["T"], meta["K_A"], meta["K_B"], meta["IN_pad"] // P, HID, OUT,
        meta["NPC"], meta["NG"], meta["WA"], meta["WB_off"],
    )
    res = run_bass_kernel_spmd(
        nc,
        in_maps,
        core_ids=list(range(N_CORES)),
        trace=trace,
        trace_cores=trace_cores,
    )
    outs = [res.results[c]["out"] for c in range(N_CORES)]
    return _assemble(outs, meta, OUT), res


def kernel(x, edge_index, W1, b1, W2, b2):
    full, _ = run(x, edge_index, W1, b1, W2, b2, trace=False)
    return full
